# revision 3
# baseline (speedup 1.0000x reference)
"""DWT (db4) kernel for Trainium2, 8 NeuronCores.

The reference computes y = x @ W (W a banded db4 decomposition matrix built
transposed) followed by an even/odd column deinterleave into [a | d].
Mathematically this is a pair of 4-tap FIR filters with stride 2 and
periodic wrap-around:

    a[p] = c0*x[2p] + c1*x[2p+1] + c2*x[2p+2] + c3*x[2p+3]
    d[p] = c3*x[2p] - c2*x[2p+1] + c1*x[2p+2] - c0*x[2p+3]
    out  = [a | d]            (indices mod N)

Sharding: batch 512 -> 4 groups of 128 (full SBUF partition dim), and the
signal into 2 halves of 2048 (+2 halo columns) -> 8 shards. Each core:
one [128, 2050] load, six fused scalar_tensor_tensor ops + two ACT scales,
one [128, 2048] store. The host (numpy) does the halo/wrap slicing and the
final reassembly.

Factored FIR (all ratios < 1 in magnitude for fp32 health):
    XE0/XE1, XO0/XO1 = even/odd stride-2 views of the shard, shift 0/+1
    u1 = XE0 + (c2/c0)*XE1        v1 = XO0 + (c3/c1)*XO1
    u2 = XE1 + (c3/c1)*XE0        v2 = XO1 + (c2/c0)*XO0
    a  = c1*((c0/c1)*u1 + v1)     d  = -c1*((c0/c1)*v2 - u2)
"""

import numpy as np

C0 = 0.4829629131445341
C1 = 0.8365163037378079
C2 = 0.2241438680420134
C3 = -0.1294095225512604

R20 = C2 / C0
R31 = C3 / C1
R01 = C0 / C1

N_CORES = 8
B, N = 512, 4096
HB = 128          # batch rows per core
HS = 2048         # signal columns per core (before halo)
HQ = 1024         # a/d outputs per core

_prog_cache = {}


def _build_program():
    import concourse.tile as tile
    from concourse import bacc, mybir

    f32 = mybir.dt.float32
    Alu = mybir.AluOpType

    nc = bacc.Bacc("TRN2", debug=False, num_devices=N_CORES)
    xs = nc.dram_tensor("xs", [HB, HS + 2], f32, kind="ExternalInput").ap()
    ys = nc.dram_tensor("ys", [HB, HS], f32, kind="ExternalOutput").ap()

    with tile.TileContext(nc) as tc:
        with tc.tile_pool(name="p", bufs=1) as pool:
            T = pool.tile([HB, HS + 2], f32)
            nc.sync.dma_start(T[:], xs[:])

            XE0 = T[:, 0:HS:2]
            XO0 = T[:, 1:HS:2]
            XE1 = T[:, 2:HS + 2:2]
            XO1 = T[:, 3:HS + 2:2]

            u1 = pool.tile([HB, HQ], f32)
            v1 = pool.tile([HB, HQ], f32)
            u2 = pool.tile([HB, HQ], f32)
            v2 = pool.tile([HB, HQ], f32)
            ap_ = pool.tile([HB, HQ], f32)
            dp_ = pool.tile([HB, HQ], f32)
            O = pool.tile([HB, HS], f32)

            stt = nc.vector.scalar_tensor_tensor
            stt(u1[:], XE1, R20, XE0, Alu.mult, Alu.add)
            stt(v1[:], XO1, R31, XO0, Alu.mult, Alu.add)
            stt(u2[:], XE0, R31, XE1, Alu.mult, Alu.add)
            stt(v2[:], XO0, R20, XO1, Alu.mult, Alu.add)
            stt(ap_[:], u1[:], R01, v1[:], Alu.mult, Alu.add)
            stt(dp_[:], v2[:], R01, u2[:], Alu.mult, Alu.subtract)

            nc.scalar.mul(O[:, 0:HQ], ap_[:], C1)
            nc.scalar.mul(O[:, HQ:HS], dp_[:], -C1)

            nc.sync.dma_start(ys[:], O[:])
    nc.compile()
    return nc


def _get_program():
    if "nc" not in _prog_cache:
        _prog_cache["nc"] = _build_program()
    return _prog_cache["nc"]


def make_shards(x: np.ndarray) -> list[np.ndarray]:
    xg = np.concatenate([x, x[:, 0:2]], axis=1)  # periodic wrap halo
    shards = []
    for c in range(N_CORES):
        g, h = c // 2, c % 2
        shards.append(
            np.ascontiguousarray(xg[HB * g:HB * (g + 1), HS * h:HS * h + HS + 2])
        )
    return shards


def assemble(outs: list[np.ndarray]) -> np.ndarray:
    out = np.empty((B, N), dtype=np.float32)
    for c in range(N_CORES):
        g, h = c // 2, c % 2
        o = outs[c]
        rows = slice(HB * g, HB * (g + 1))
        out[rows, HQ * h:HQ * h + HQ] = o[:, 0:HQ]
        out[rows, HQ * 2 + HQ * h:HQ * 2 + HQ * h + HQ] = o[:, HQ:HS]
    return out


def run_on_device(x: np.ndarray, trace: bool = False):
    from concourse import bass_utils

    nc = _get_program()
    in_maps = [{"xs": s} for s in make_shards(x)]
    res = bass_utils.run_bass_kernel_spmd(
        nc, in_maps, core_ids=list(range(N_CORES)), trace=trace
    )
    out = assemble([res.results[c]["ys"] for c in range(N_CORES)])
    return out, res


def kernel(input, w=None, **_ignored):
    x = np.asarray(input, dtype=np.float32)
    assert x.shape == (B, N), x.shape
    out, _ = run_on_device(x)
    return out


# revision 7
# speedup vs baseline: 1.7634x; 1.7634x over previous
"""DWT (db4) kernel for Trainium2, 8 NeuronCores.

The reference computes y = x @ W (W a banded db4 decomposition matrix built
transposed) followed by an even/odd column deinterleave into [a | d].
Mathematically this is a pair of 4-tap FIR filters with stride 2 and
periodic wrap-around:

    a[p] = c0*x[2p] + c1*x[2p+1] + c2*x[2p+2] + c3*x[2p+3]
    d[p] = c3*x[2p] - c2*x[2p+1] + c1*x[2p+2] - c0*x[2p+3]
    out  = [a | d]            (indices mod N)

Sharding: batch 512 -> 4 groups of 128 (full SBUF partition dim), and the
signal into 2 halves of 2048 (+2 halo columns) -> 8 shards. Each core:
one [128, 2050] load, six fused scalar_tensor_tensor ops + two ACT scales,
one [128, 2048] store. The host (numpy) does the halo/wrap slicing and the
final reassembly.

Factored FIR (all ratios < 1 in magnitude for fp32 health):
    XE0/XE1, XO0/XO1 = even/odd stride-2 views of the shard, shift 0/+1
    u1 = XE0 + (c2/c0)*XE1        v1 = XO0 + (c3/c1)*XO1
    u2 = XE1 + (c3/c1)*XE0        v2 = XO1 + (c2/c0)*XO0
    a  = c1*((c0/c1)*u1 + v1)     d  = -c1*((c0/c1)*v2 - u2)
"""

import numpy as np

C0 = 0.4829629131445341
C1 = 0.8365163037378079
C2 = 0.2241438680420134
C3 = -0.1294095225512604

R20 = C2 / C0
R31 = C3 / C1
R01 = C0 / C1

N_CORES = 8
B, N = 512, 4096
HB = 128          # batch rows per core
HS = 2048         # signal columns per core (before halo)
HQ = 1024         # a/d outputs per core

_prog_cache = {}


# Uneven chunks: a small first chunk lets DVE start while the rest streams in.
CHUNK_Q = [256, 768]     # outputs per chunk (a and d each); sums to HQ


def _build_program():
    import concourse.tile as tile
    from concourse import bacc, mybir

    f32 = mybir.dt.float32
    Alu = mybir.AluOpType

    nc = bacc.Bacc("TRN2", debug=False, num_devices=N_CORES)
    xs = nc.dram_tensor("xs", [HB, HS + 2], f32, kind="ExternalInput").ap()
    ys = nc.dram_tensor("ys", [HB, HS], f32, kind="ExternalOutput").ap()

    with tile.TileContext(nc) as tc:
        with tc.tile_pool(name="p", bufs=1) as pool:
            # All input chunk loads issued up front so the DMA heads overlap.
            Ts = []
            q0 = 0
            for c, cq in enumerate(CHUNK_Q):
                cs = 2 * cq
                T = pool.tile([HB, cs + 2], f32, tag=f"T{c}")
                nc.sync.dma_start(T[:], xs[:, 2 * q0:2 * q0 + cs + 2])
                Ts.append(T)
                q0 += cq

            stt = nc.vector.scalar_tensor_tensor
            q0 = 0
            for c, cq in enumerate(CHUNK_Q):
                cs = 2 * cq
                T = Ts[c]
                XE0 = T[:, 0:cs:2]
                XO0 = T[:, 1:cs:2]
                XE1 = T[:, 2:cs + 2:2]
                XO1 = T[:, 3:cs + 2:2]

                # U = [u1 | -v2], V = [v1 | u2]; then one fused combine:
                #   [a_pre | d_pre] = R01*U + V, and one uniform *C1 scale:
                #   a = C1*(R01*u1 + v1),  d = C1*(u2 - R01*v2)
                U = pool.tile([HB, 2 * cq], f32, tag=f"U{c}")
                V = pool.tile([HB, 2 * cq], f32, tag=f"V{c}")
                AD = pool.tile([HB, 2 * cq], f32, tag=f"AD{c}")
                O = pool.tile([HB, 2 * cq], f32, tag=f"O{c}")

                stt(U[:, 0:cq], XE1, R20, XE0, Alu.mult, Alu.add)        # u1
                stt(V[:, 0:cq], XO1, R31, XO0, Alu.mult, Alu.add)        # v1
                stt(V[:, cq:2 * cq], XE0, R31, XE1, Alu.mult, Alu.add)   # u2
                stt(U[:, cq:2 * cq], XO0, -R20, XO1, Alu.mult, Alu.subtract)  # -v2
                stt(AD[:], U[:], R01, V[:], Alu.mult, Alu.add)
                nc.scalar.mul(O[:], AD[:], C1)

                nc.sync.dma_start(ys[:, q0:q0 + cq], O[:, 0:cq])
                nc.sync.dma_start(ys[:, HQ + q0:HQ + q0 + cq], O[:, cq:2 * cq])
                q0 += cq
    nc.compile()
    return nc


def _get_program():
    if "nc" not in _prog_cache:
        _prog_cache["nc"] = _build_program()
    return _prog_cache["nc"]


def make_shards(x: np.ndarray) -> list[np.ndarray]:
    xg = np.concatenate([x, x[:, 0:2]], axis=1)  # periodic wrap halo
    shards = []
    for c in range(N_CORES):
        g, h = c // 2, c % 2
        shards.append(
            np.ascontiguousarray(xg[HB * g:HB * (g + 1), HS * h:HS * h + HS + 2])
        )
    return shards


def assemble(outs: list[np.ndarray]) -> np.ndarray:
    out = np.empty((B, N), dtype=np.float32)
    for c in range(N_CORES):
        g, h = c // 2, c % 2
        o = outs[c]
        rows = slice(HB * g, HB * (g + 1))
        out[rows, HQ * h:HQ * h + HQ] = o[:, 0:HQ]
        out[rows, HQ * 2 + HQ * h:HQ * 2 + HQ * h + HQ] = o[:, HQ:HS]
    return out


def run_on_device(x: np.ndarray, trace: bool = False):
    from concourse import bass_utils

    nc = _get_program()
    in_maps = [{"xs": s} for s in make_shards(x)]
    res = bass_utils.run_bass_kernel_spmd(
        nc, in_maps, core_ids=list(range(N_CORES)), trace=trace
    )
    out = assemble([res.results[c]["ys"] for c in range(N_CORES)])
    return out, res


def kernel(input, w=None, **_ignored):
    x = np.asarray(input, dtype=np.float32)
    assert x.shape == (B, N), x.shape
    out, _ = run_on_device(x)
    return out
